# revision 32
# baseline (speedup 1.0000x reference)
"""Trainium2 Bass kernel for the masked note-accuracy loss.

Reference math (per sequence n):
    pred      = (sigmoid(x) > 0.5) = (x > 0)
    S_n       = sum_{t,d} pred * target                     (tru_pos)
    A[n,t]    = false_pos + false_neg = sum_d |pred - target|
    ratio     = S_n / (S_n + A[n,t]) = 2S_n / (2S_n + 2A[n,t])
    acc_n     = sum_{t<T_n} ratio / T_n,   T_n = sum_t mask[n,t]
    out       = sum_n acc_n

Sharding: data-parallel over N=128 sequences -> 16 per core on 8 cores;
the host sums the 8 per-core partial scalars.

Per-core pipeline, one sequence per step ([T,D] loaded as a [128,16,88]
tile, t = p*16+k, 5632B contiguous per partition; x via the SP HWDGE
queue, target via the ACT queue, mask once via SWDGE):
  DVE pass1: V = (x>0) - target (bf16), accum_out = per-partition (P-Q)
  DVE pass2: -A[t] = negated segmented abs-reduce of V over d
  ACT pass : Copy(2*target) with accum_out = per-partition 2Q
  (separate stats tiles per writer engine -- a shared tile would
   serialize ACT behind DVE on tile-granular WAW tracking)
  mini epilogue (tiny; hidden under later sequences' DMAs):
    PE colsums -> one psum row [-A(16) | P-Q | 2Q]; its full reduce is
    directly 2S (sign trick).  PE broadcasts 2S to all partitions;
    den = -2*(-A) + 2S;  rat = recip(den) * 2S * mask;  PE colsum;
    acc += sum_t(rat) / T_n.
Final: one 4-byte DMA of the accumulated scalar.

Modeled (TimelineSim cost model) at 76.0 us/core vs the 65.0 us
HBM-stream roofline (23.2 MB/core at 360 GB/s); the gap is the Tile
lead-in/tail barriers, the last sequence's exposed DVE passes, and
the final reduction chain.
"""

import numpy as np

import concourse.bacc as bacc
import concourse.tile as tile
from concourse import mybir
from concourse.alu_op_type import AluOpType
from concourse.bass_utils import run_bass_kernel_spmd

N, T, D = 128, 2048, 88
N_CORES = 8
NS = N // N_CORES
P = 128
K = T // P

_cached_nc = None

USE_BF16_V = True
USE_NEGATE = True


def _build():
    f32 = mybir.dt.float32
    vdt = mybir.dt.bfloat16 if USE_BF16_V else f32
    nc = bacc.Bacc("TRN2", target_bir_lowering=False, debug=False,
                   num_devices=N_CORES)
    xd = nc.dram_tensor("output", [NS, T, D], f32, kind="ExternalInput")
    yd = nc.dram_tensor("target", [NS, T, D], f32, kind="ExternalInput")
    md = nc.dram_tensor("mask", [NS, T], mybir.dt.int32, kind="ExternalInput")
    od = nc.dram_tensor("partial", [1, 1], f32, kind="ExternalOutput")

    AX = mybir.AxisListType.X

    with tile.TileContext(nc) as tc:
        with (
            tc.tile_pool(name="data", bufs=3) as data_pool,
            tc.tile_pool(name="work", bufs=2) as work_pool,
            tc.tile_pool(name="mini", bufs=2) as mini_pool,
            tc.tile_pool(name="singles", bufs=1) as singles,
            tc.tile_pool(name="psl", bufs=2, space="PSUM") as psum_loop,
            tc.tile_pool(name="psk", bufs=1, space="PSUM") as psum_keep,
        ):
            stA = singles.tile([P, NS, 16], f32)
            stPQ = singles.tile([P, NS], f32)
            stQ2 = singles.tile([P, NS], f32)
            maskf = singles.tile([P, NS, K], f32)
            maski = singles.tile([P, NS, K], mybir.dt.int32)
            ones_col = singles.tile([P, 1], f32)
            ones_row = singles.tile([1, P], f32)
            inv_ti = singles.tile([1, NS], f32)
            row_ti = singles.tile([1, NS], f32)
            nc.vector.memset(ones_col[:], 1.0)
            nc.vector.memset(ones_row[:], 1.0)

            ps_m = psum_keep.tile([1, NS * K], f32)

            acc0 = mini_pool.tile([1, 1], f32, tag="acc")
            acc_prev = [acc0]
            nc.vector.memset(acc_prev[0][:], 0.0)

            def load(n):
                xt = data_pool.tile([P, K, D], f32, tag="xt")
                yt = data_pool.tile([P, K, D], f32, tag="yt")
                nc.sync.dma_start(xt[:], xd.ap()[n].rearrange("(p k) d -> p k d", p=P))
                nc.scalar.dma_start(yt[:], yd.ap()[n].rearrange("(p k) d -> p k d", p=P))
                return xt, yt

            def compute(n, xt, yt):
                v = work_pool.tile([P, K, D], vdt, tag="v")
                nc.vector.scalar_tensor_tensor(
                    out=v[:], in0=xt[:], scalar=0.0, in1=yt[:],
                    op0=AluOpType.is_gt, op1=AluOpType.subtract,
                    accum_out=stPQ[:, n : n + 1],
                )
                nc.vector.tensor_reduce(
                    out=stA[:, n, :], in_=v[:], axis=AX, op=AluOpType.add,
                    apply_absolute_value=True, negate=USE_NEGATE,
                )
                scratch = work_pool.tile([P, K, D], vdt, tag="scratch")
                nc.scalar.activation(
                    out=scratch[:], in_=yt[:],
                    func=mybir.ActivationFunctionType.Copy, scale=2.0,
                    accum_out=stQ2[:, n : n + 1],
                )
                # ---- mini epilogue ----
                ps_st = psum_loop.tile([1, 18], f32, tag="ps_st")
                nc.tensor.matmul(ps_st[0:1, 0:16], ones_col[:], stA[:, n, :])
                nc.tensor.matmul(ps_st[0:1, 16:17], ones_col[:],
                                 stPQ[:, n : n + 1])
                nc.tensor.matmul(ps_st[0:1, 17:18], ones_col[:],
                                 stQ2[:, n : n + 1])
                row_s2 = mini_pool.tile([1, 1], f32, tag="row_s2")
                nc.vector.tensor_reduce(
                    out=row_s2[:], in_=ps_st[:], axis=AX, op=AluOpType.add)
                ps_b = psum_loop.tile([P, 1], f32, tag="ps_b")
                nc.tensor.matmul(ps_b[:], ones_row[:], row_s2[:])
                sb_b = ps_b
                den = mini_pool.tile([P, K], f32, tag="den")
                nc.vector.tensor_scalar(
                    out=den[:], in0=stA[:, n, :],
                    scalar1=-2.0, scalar2=sb_b[:], op0=AluOpType.mult,
                    op1=AluOpType.add)
                rec = mini_pool.tile([P, K], f32, tag="rec")
                nc.vector.reciprocal(rec[:], den[:])
                rat = mini_pool.tile([P, K], f32, tag="rat")
                nc.vector.scalar_tensor_tensor(
                    out=rat[:], in0=rec[:], scalar=sb_b[:],
                    in1=maskf[:, n, :],
                    op0=AluOpType.mult, op1=AluOpType.mult)
                ps_rat = psum_loop.tile([1, K], f32, tag="ps_rat")
                nc.tensor.matmul(ps_rat[:], ones_col[:], rat[:])
                row_c = mini_pool.tile([1, 1], f32, tag="row_c")
                nc.vector.tensor_reduce(
                    out=row_c[:], in_=ps_rat[:], axis=AX, op=AluOpType.add)
                acc_new = mini_pool.tile([1, 1], f32, tag="acc")
                nc.vector.scalar_tensor_tensor(
                    out=acc_new[:], in0=row_c[:],
                    scalar=inv_ti[0:1, n : n + 1], in1=acc_prev[0][:],
                    op0=AluOpType.mult, op1=AluOpType.add)
                acc_prev[0] = acc_new

            xt0, yt0 = load(0)
            nc.gpsimd.dma_start(maski[:], md.ap().rearrange("n (p k) -> p n k", p=P))
            nc.vector.tensor_copy(maskf[:], maski[:])
            nc.tensor.matmul(ps_m[:], ones_col[:],
                             maskf[:].rearrange("p a b -> p (a b)"))
            nc.vector.tensor_reduce(
                out=row_ti[:], in_=ps_m[:].rearrange("o (a b) -> o a b", a=NS),
                axis=AX, op=AluOpType.add)
            nc.vector.reciprocal(inv_ti[:], row_ti[:])

            compute(0, xt0, yt0)
            for n in range(1, NS):
                xt, yt = load(n)
                compute(n, xt, yt)

            nc.sync.dma_start(od.ap(), acc_prev[0][:])

    nc.compile()
    return nc


def kernel(output, target, mask):
    global _cached_nc
    if _cached_nc is None:
        _cached_nc = _build()
    nc = _cached_nc
    output = np.asarray(output, dtype=np.float32)
    target = np.asarray(target, dtype=np.float32)
    mask = np.asarray(mask, dtype=np.int32)
    in_maps = []
    for c in range(N_CORES):
        sl = slice(c * NS, (c + 1) * NS)
        in_maps.append({
            "output": np.ascontiguousarray(output[sl]),
            "target": np.ascontiguousarray(target[sl]),
            "mask": np.ascontiguousarray(mask[sl]),
        })
    res = run_bass_kernel_spmd(nc, in_maps, list(range(N_CORES)))
    total = np.float32(0.0)
    for c in range(N_CORES):
        total = np.float32(total + np.float32(res.results[c]["partial"].reshape(())))
    return np.float32(total)


# revision 40
# speedup vs baseline: 1.0038x; 1.0038x over previous
"""Trainium2 Bass kernel for the masked note-accuracy loss.

Reference math (per sequence n):
    pred      = (sigmoid(x) > 0.5) = (x > 0)
    S_n       = sum_{t,d} pred * target                     (tru_pos)
    A[n,t]    = false_pos + false_neg = sum_d |pred - target|
    ratio     = S_n / (S_n + A[n,t]) = 2S_n / (2S_n + 2A[n,t])
    acc_n     = sum_{t<T_n} ratio / T_n,   T_n = sum_t mask[n,t]
    out       = sum_n acc_n

Sharding: data-parallel over N=128 sequences -> 16 per core on 8 cores;
the host sums the 8 per-core partial scalars.

Per-core pipeline, one sequence per step ([T,D] loaded as a [128,16,88]
tile, t = p*16+k, 5632B contiguous per partition; x via the SP HWDGE
queue, target via the ACT queue, mask once via SWDGE):
  DVE pass1: V = (x>0) - target (bf16), accum_out = per-partition (P-Q)
  DVE pass2: -A[t] = negated segmented abs-reduce of V over d
  ACT pass : Copy(2*target) with accum_out = per-partition 2Q
  (separate stats tiles per writer engine -- a shared tile would
   serialize ACT behind DVE on tile-granular WAW tracking)
  mini epilogue (tiny; hidden under later sequences' DMAs):
    PE colsums -> one psum row [-A(16) | P-Q | 2Q]; its full reduce is
    directly 2S (sign trick).  PE broadcasts 2S to all partitions;
    den = -2*(-A) + 2S;  rat = recip(den) * 2S * mask;  PE colsum;
    acc += sum_t(rat) / T_n.
Final: one 4-byte DMA of the accumulated scalar.

Modeled (TimelineSim cost model) at 76.0 us/core vs the 65.0 us
HBM-stream roofline (23.2 MB/core at 360 GB/s); the gap is the Tile
lead-in/tail barriers, the last sequence's exposed DVE passes, and
the final reduction chain.
"""

import numpy as np

import concourse.bacc as bacc
import concourse.tile as tile
from concourse import mybir
from concourse.alu_op_type import AluOpType
from concourse.bass_utils import run_bass_kernel_spmd

N, T, D = 128, 2048, 88
N_CORES = 8
NS = N // N_CORES
P = 128
K = T // P

_cached_nc = None

USE_BF16_V = True
USE_NEGATE = True


def _build():
    f32 = mybir.dt.float32
    vdt = mybir.dt.bfloat16 if USE_BF16_V else f32
    nc = bacc.Bacc("TRN2", target_bir_lowering=False, debug=False,
                   num_devices=N_CORES)
    xd = nc.dram_tensor("output", [NS, T, D], f32, kind="ExternalInput")
    yd = nc.dram_tensor("target", [NS, T, D], f32, kind="ExternalInput")
    md = nc.dram_tensor("mask", [NS, T], mybir.dt.int32, kind="ExternalInput")
    od = nc.dram_tensor("partial", [1, 1], f32, kind="ExternalOutput")

    AX = mybir.AxisListType.X

    with tile.TileContext(nc) as tc:
        with (
            tc.tile_pool(name="data", bufs=3) as data_pool,
            tc.tile_pool(name="work", bufs=2) as work_pool,
            tc.tile_pool(name="mini", bufs=2) as mini_pool,
            tc.tile_pool(name="singles", bufs=1) as singles,
            tc.tile_pool(name="psl", bufs=2, space="PSUM") as psum_loop,
            tc.tile_pool(name="psk", bufs=1, space="PSUM") as psum_keep,
        ):
            stA = singles.tile([P, NS, 16], f32)
            stPQ = singles.tile([P, NS], f32)
            stQ2 = singles.tile([P, NS], f32)
            maskf = singles.tile([P, NS, K], f32)
            maski = singles.tile([P, NS, K], mybir.dt.int32)
            ones128 = singles.tile([P, P], f32)
            inv_ti = singles.tile([1, NS], f32)
            row_ti = singles.tile([1, NS], f32)
            nc.vector.memset(ones128[:], 1.0)

            ps_m = psum_keep.tile([1, NS * K], f32)

            acc0 = mini_pool.tile([1, 1], f32, tag="acc")
            acc_prev = [acc0]
            nc.vector.memset(acc_prev[0][:], 0.0)

            def load(n):
                xt = data_pool.tile([P, K, D], f32, tag="xt")
                yt = data_pool.tile([P, K, D], f32, tag="yt")
                nc.sync.dma_start(xt[:], xd.ap()[n].rearrange("(p k) d -> p k d", p=P))
                nc.scalar.dma_start(yt[:], yd.ap()[n].rearrange("(p k) d -> p k d", p=P))
                return xt, yt

            def compute(n, xt, yt):
                v = work_pool.tile([P, K, D], vdt, tag="v")
                p1 = nc.vector.scalar_tensor_tensor(
                    out=v[:], in0=xt[:], scalar=0.0, in1=yt[:],
                    op0=AluOpType.is_gt, op1=AluOpType.subtract,
                    accum_out=stPQ[:, n : n + 1],
                )
                nc.vector.tensor_reduce(
                    out=stA[:, n, :], in_=v[:], axis=AX, op=AluOpType.add,
                    apply_absolute_value=True, negate=USE_NEGATE,
                )
                scratch = work_pool.tile([P, K, D], vdt, tag="scratch")
                nc.scalar.activation(
                    out=scratch[:], in_=yt[:],
                    func=mybir.ActivationFunctionType.Copy, scale=2.0,
                    accum_out=stQ2[:, n : n + 1],
                )
                # ---- mini epilogue ----
                # all-ones [128,128] stationary: the colsum matmul lands the
                # [-A | P-Q | 2Q] sums on EVERY partition, so 2S is just a
                # per-partition free-dim reduce -- no broadcast round-trip.
                ps_st = psum_loop.tile([P, 18], f32, tag="ps_st")
                nc.tensor.matmul(ps_st[:, 0:16], ones128[:], stA[:, n, :])
                nc.tensor.matmul(ps_st[:, 16:17], ones128[:],
                                 stPQ[:, n : n + 1])
                nc.tensor.matmul(ps_st[:, 17:18], ones128[:],
                                 stQ2[:, n : n + 1])
                s2p = mini_pool.tile([P, 1], f32, tag="s2p")
                nc.vector.tensor_reduce(
                    out=s2p[:], in_=ps_st[:], axis=AX, op=AluOpType.add)
                den = mini_pool.tile([P, K], f32, tag="den")
                nc.vector.tensor_scalar(
                    out=den[:], in0=stA[:, n, :],
                    scalar1=-2.0, scalar2=s2p[:], op0=AluOpType.mult,
                    op1=AluOpType.add)
                rec = mini_pool.tile([P, K], f32, tag="rec")
                nc.vector.reciprocal(rec[:], den[:])
                rat = mini_pool.tile([P, K], f32, tag="rat")
                nc.vector.scalar_tensor_tensor(
                    out=rat[:], in0=rec[:], scalar=s2p[:],
                    in1=maskf[:, n, :],
                    op0=AluOpType.mult, op1=AluOpType.mult)
                ps_rat = psum_loop.tile([P, K], f32, tag="ps_rat")
                nc.tensor.matmul(ps_rat[:], ones128[:], rat[:])
                row_c = mini_pool.tile([1, 1], f32, tag="row_c")
                nc.vector.tensor_reduce(
                    out=row_c[:], in_=ps_rat[0:1, :], axis=AX, op=AluOpType.add)
                acc_new = mini_pool.tile([1, 1], f32, tag="acc")
                nc.vector.scalar_tensor_tensor(
                    out=acc_new[:], in0=row_c[:],
                    scalar=inv_ti[0:1, n : n + 1], in1=acc_prev[0][:],
                    op0=AluOpType.mult, op1=AluOpType.add)
                acc_prev[0] = acc_new

            xt0, yt0 = load(0)
            nc.gpsimd.dma_start(maski[:], md.ap().rearrange("n (p k) -> p n k", p=P))
            nc.vector.tensor_copy(maskf[:], maski[:])
            nc.tensor.matmul(ps_m[:], ones128[:, 0:1],
                             maskf[:].rearrange("p a b -> p (a b)"))
            nc.vector.tensor_reduce(
                out=row_ti[:], in_=ps_m[:].rearrange("o (a b) -> o a b", a=NS),
                axis=AX, op=AluOpType.add)
            nc.vector.reciprocal(inv_ti[:], row_ti[:])

            compute(0, xt0, yt0)
            for n in range(1, NS):
                xt, yt = load(n)
                compute(n, xt, yt)

            nc.sync.dma_start(od.ap(), acc_prev[0][:])

    nc.compile()
    return nc


def kernel(output, target, mask):
    global _cached_nc
    if _cached_nc is None:
        _cached_nc = _build()
    nc = _cached_nc
    output = np.asarray(output, dtype=np.float32)
    target = np.asarray(target, dtype=np.float32)
    mask = np.asarray(mask, dtype=np.int32)
    in_maps = []
    for c in range(N_CORES):
        sl = slice(c * NS, (c + 1) * NS)
        in_maps.append({
            "output": np.ascontiguousarray(output[sl]),
            "target": np.ascontiguousarray(target[sl]),
            "mask": np.ascontiguousarray(mask[sl]),
        })
    res = run_bass_kernel_spmd(nc, in_maps, list(range(N_CORES)))
    total = np.float32(0.0)
    for c in range(N_CORES):
        total = np.float32(total + np.float32(res.results[c]["partial"].reshape(())))
    return np.float32(total)
